# revision 1
# baseline (speedup 1.0000x reference)
"""Trainium2 Bass kernel for nn_CNN_MAMBA2 (CNN + Mamba2(L=1) + MLP head).

Strategy: pure data parallel over batch (B=256 -> 32 per core x 8 cores).
Each core runs the full network on its batch shard; weights are replicated.

Layouts (per core, bh = 32 batches x 2 rows = 64 independent 1D signals):
  X    [64, 3936]   batch-major padded input (xpad[i] = x[i-25])
  Xp   [128, 7680]  position-major: Xp[p, 64*C+bh] = xpad_bh[32*C+p]
                    (built with 120 PE transposes of overlapping 128-col blocks)
  conv1: out w = 8C + j + 4*delta; lhsT packs (tap k, delta) into K=67;
         4 j-groups x 15 N=512 chunks of fp32r matmuls; maxpool(4) fused as
         DVE max over the 4 j-group PSUMs; BN+ReLU fused into evacuation.
  P1   [128, 8320]  pooled, partition = 64*delta + ci, col = (C+5)*64 + bh
                    where pooled position m = 2C + delta  (5 C-blocks zero pad)
  conv2: tap pairs (2j, 2j+1) land on the two delta halves -> K=128 packed,
         11 accumulating matmuls per N=512 chunk.
  C3in [128, 8192]  conv2 out, col = (w+4)*64 + bh (4 w-blocks zero pad)
  conv3: K=128 per tap, 9 taps x 2 co-halves, N<=512 chunks.
  H3   2 x [128, 3840]  conv3 out (v, bh); avgpool -> feature-major h [256, 32]
  Mamba2 with L=1: single scan step from h0=0 =>
         y = xin * (dt * (B.C) + D) (per head), gated RMSNorm, out_proj, MLP.
  Feature-major mamba; partition reductions/broadcasts via ones-matmuls.

Host-side prep is layout-only (transpose/reshape/pad/tile of weights); all
arithmetic (BN folding, silu, conv, matmuls, norms) happens on device.
"""

import numpy as np

import bass_rust
import concourse.bass as bass
import concourse.mybir as mybir
from concourse import masks
from concourse.tile import TileContext
from concourse.bass_utils import run_bass_kernel_spmd

F32 = mybir.dt.float32
F32R = mybir.dt.float32r
AF = mybir.ActivationFunctionType
ALU = mybir.AluOpType
AX = mybir.AxisListType

EPS = 1e-5
NCORES = 8
BSH = 32            # batches per core
BH = 64             # bh signals per core
NC1 = 120           # C blocks (conv1 output pairs / pool blocks)
XPAD = 3936


def _split_multi_waits(nc):
    """This walrus build accepts at most one sync-wait command per
    instruction; Tile's sem assignment attaches several. Hoist extra waits
    onto dedicated single-wait nops right before the instruction (same
    engine), which preserves blocking semantics."""
    n = 0
    for fn in nc.m.functions:
        for bb in fn.blocks:
            out = []
            for inst in bb.instructions:
                si = inst.sync_info
                waits = list(si.on_wait) if si is not None else []
                if len(waits) > 1:
                    for w in waits[:-1]:
                        n += 1
                        nop = mybir.InstNoOp(name=f"waitnop-{n}", ins=[], outs=[])
                        nop.engine = inst.engine
                        nop.debug = inst.debug
                        nop.sync_info = bass_rust.SyncInfo(
                            on_wait=[w], on_update=[]
                        )
                        out.append(nop)
                    si.on_wait = [waits[-1]]
                    inst.sync_info = si
                out.append(inst)
            bb.instructions = out


# --------------------------------------------------------------------------
# host-side weight layout prep (layout only: transpose / reshape / pad / tile)
# --------------------------------------------------------------------------

def _prep_weights(inp):
    f32 = np.float32
    c1w = np.asarray(inp["c1w"], f32).reshape(64, 51)
    # lhsT for conv1: K rows are input positions c relative to the 32-position
    # chunk base; column m = 128*j is absorbed by leading 4j zero rows so the
    # rhs can always start at partition 0 (PE base-partition constraint).
    w1t = np.zeros((79, 4, 128), f32)
    for j in range(4):
        for d in range(2):
            for c in range(4 * j + 16 * d, 4 * j + 16 * d + 51):
                w1t[c, j, 64 * d : 64 * d + 64] = c1w[:, c - 4 * j - 16 * d]
    w1t = w1t.reshape(79, 512)

    c2w = np.asarray(inp["c2w"], f32).reshape(128, 64, 21)
    w2t = np.zeros((128, 11, 128), f32)
    for jp in range(11):
        for d in range(2):
            t = 2 * jp + d
            if t <= 20:
                w2t[64 * d : 64 * d + 64, jp, :] = c2w[:, :, t].T

    c3w = np.asarray(inp["c3w"], f32).reshape(256, 128, 9)
    w3t = np.zeros((128, 2, 9, 128), f32)
    for hf in range(2):
        for k in range(9):
            w3t[:, hf, k, :] = c3w[128 * hf : 128 * hf + 128, :, k].T

    mw_in = np.asarray(inp["mw_in"], f32)          # [1160, 256]
    w_inT = np.zeros((128, 2, 1160), f32)
    for k in range(2):
        w_inT[:, k, :] = mw_in[:, 128 * k : 128 * k + 128].T

    mw_out = np.asarray(inp["mw_out"], f32)        # [256, 512]
    w_outT = np.zeros((128, 4, 2, 128), f32)
    for k in range(4):
        for m in range(2):
            w_outT[:, k, m, :] = mw_out[
                128 * m : 128 * m + 128, 128 * k : 128 * k + 128
            ].T

    f1w = np.asarray(inp["f1w"], f32)              # [64, 256]
    f1wT = np.zeros((128, 2, 64), f32)
    for k in range(2):
        f1wT[:, k, :] = f1w[:, 128 * k : 128 * k + 128].T

    f2wT = np.asarray(inp["f2w"], f32).reshape(1, 64).T.copy()   # [64, 1]

    def t2(a):
        return np.tile(np.asarray(a, f32), 2)

    def pd(a):
        a = np.asarray(a, f32)
        return np.pad(a, (0, 128 - a.shape[0]))

    vecs = np.zeros((128, 44), f32)
    # cols 0-4 bn gammas, 5-9 betas, 10-14 means, 15-19 vars, 20-24 pre-bias
    vecs[:, 0] = t2(inp["bn1g"]); vecs[:, 5] = t2(inp["bn1b"])
    vecs[:, 10] = t2(inp["bn1m"]); vecs[:, 15] = t2(inp["bn1v"])
    vecs[:, 20] = t2(inp["c1b"])
    vecs[:, 1] = inp["bn2g"]; vecs[:, 6] = inp["bn2b"]
    vecs[:, 11] = inp["bn2m"]; vecs[:, 16] = inp["bn2v"]
    vecs[:, 21] = inp["c2b"]
    for hf in range(2):
        s = slice(128 * hf, 128 * hf + 128)
        vecs[:, 2 + hf] = inp["bn3g"][s]; vecs[:, 7 + hf] = inp["bn3b"][s]
        vecs[:, 12 + hf] = inp["bn3m"][s]; vecs[:, 17 + hf] = inp["bn3v"][s]
        vecs[:, 22 + hf] = inp["c3b"][s]
    vecs[:, 4] = pd(inp["bn4g"]); vecs[:, 9] = pd(inp["bn4b"])
    vecs[:, 14] = pd(inp["bn4m"]); vecs[:, 19] = pd(inp["bn4v"])
    vecs[:, 24] = pd(inp["f1b"])
    vecs[0:8, 25] = inp["mdt_bias"]
    vecs[0:8, 26] = inp["mD"]
    vecs[0:1, 27] = inp["f2b"]
    mcw = np.asarray(inp["mconv_w"], f32)[:, 0, 3]
    mcb = np.asarray(inp["mconv_b"], f32)
    vecs[:, 28:33] = mcw.reshape(5, 128).T
    vecs[:, 33:38] = mcb.reshape(5, 128).T
    vecs[:, 38:42] = np.asarray(inp["mnorm_w"], f32).reshape(4, 128).T
    vecs[0:64, 42] = mcw[576:640]
    vecs[0:64, 43] = mcb[576:640]

    # constant head-expansion matrix: emat[h, 128*t + m] = 1 iff h == 2t + m//64
    emat = np.zeros((8, 512), f32)
    for t in range(4):
        emat[2 * t, 128 * t : 128 * t + 64] = 1.0
        emat[2 * t + 1, 128 * t + 64 : 128 * t + 128] = 1.0

    return {
        "w1t": w1t, "w2t": w2t.reshape(128, -1), "w3t": w3t.reshape(128, -1),
        "w_inT": w_inT.reshape(128, -1), "w_outT": w_outT.reshape(128, -1),
        "f1wT": f1wT.reshape(128, -1), "f2wT": f2wT, "vecs": vecs, "emat": emat,
    }


# --------------------------------------------------------------------------
# device kernel
# --------------------------------------------------------------------------

def _build_nc():
    nc = bass.Bass("TRN2", target_bir_lowering=False, debug=False)

    x_d = nc.dram_tensor("x", [BSH, 2, 3840], F32, kind="ExternalInput").ap()
    w1t_d = nc.dram_tensor("w1t", [79, 512], F32R, kind="ExternalInput").ap()
    w2t_d = nc.dram_tensor("w2t", [128, 11 * 128], F32R, kind="ExternalInput").ap()
    w3t_d = nc.dram_tensor("w3t", [128, 18 * 128], F32R, kind="ExternalInput").ap()
    w_inT_d = nc.dram_tensor("w_inT", [128, 2 * 1160], F32, kind="ExternalInput").ap()
    w_outT_d = nc.dram_tensor("w_outT", [128, 1024], F32, kind="ExternalInput").ap()
    f1wT_d = nc.dram_tensor("f1wT", [128, 128], F32, kind="ExternalInput").ap()
    f2wT_d = nc.dram_tensor("f2wT", [64, 1], F32, kind="ExternalInput").ap()
    vecs_d = nc.dram_tensor("vecs", [128, 44], F32, kind="ExternalInput").ap()
    emat_d = nc.dram_tensor("emat", [8, 512], F32, kind="ExternalInput").ap()
    y_d = nc.dram_tensor("y", [1, BSH], F32, kind="ExternalOutput").ap()

    with TileContext(nc) as tc:
        _body(nc, tc, x_d, w1t_d, w2t_d, w3t_d, w_inT_d, w_outT_d,
              f1wT_d, f2wT_d, vecs_d, emat_d, y_d)
    _split_multi_waits(nc)
    return nc


def _body(nc, tc, x_d, w1t_d, w2t_d, w3t_d, w_inT_d, w_outT_d,
          f1wT_d, f2wT_d, vecs_d, emat_d, y_d):
    with (
        tc.tile_pool(name="pw", bufs=1) as pw,
        tc.tile_pool(name="pmain", bufs=1) as pm,
        tc.tile_pool(name="ptmp", bufs=3) as pt,
        tc.tile_pool(name="pp", bufs=1, space="PSUM") as pp,
    ):
        # ---- X: padded batch-major input, loaded in chunks so transposes
        # can start before the whole shard lands ----
        X = pm.tile([64, XPAD], F32)
        nc.gpsimd.memset(X[:, 0:25], 0.0)
        nc.gpsimd.memset(X[:, 3865:XPAD], 0.0)
        xflat = x_d.rearrange("b h w -> (b h) w")
        xcuts = [0, 352, 640, 1600, 2720, 3840]
        for c in range(5):
            w0, w1 = xcuts[c], xcuts[c + 1]
            nc.sync.dma_start(X[:, 25 + w0 : 25 + w1], xflat[:, w0:w1])

        ident = pw.tile([64, 64], F32)
        masks.make_identity(nc, ident[:])
        w1t = pw.tile([79, 512], F32R)
        nc.sync.dma_start(w1t[:], w1t_d)
        vecs = pw.tile([128, 44], F32)
        nc.sync.dma_start(vecs[:], vecs_d)

        # ---- T / T2: position-major via PE transposes (stride 64) ----
        # T[q, 64*D + bh] = xpad_bh[64*D + q]; T2 offset by 32 positions
        T = pm.tile([128, 60 * 64], F32R)
        T2 = pm.tile([128, 60 * 64], F32R)
        P1 = pm.tile([128, 130 * 64], F32R)
        nc.gpsimd.memset(P1[:, 0:320].bitcast(F32), 0.0)
        nc.gpsimd.memset(P1[:, 8000:8320].bitcast(F32), 0.0)
        C3in = pm.tile([128, 128 * 64], F32R)
        nc.gpsimd.memset(C3in[:, 0:256].bitcast(F32), 0.0)
        nc.gpsimd.memset(C3in[:, 7936:8192].bitcast(F32), 0.0)
        H3 = [pm.tile([128, 60 * 64], F32, tag=f"h3_{i}", name=f"h3_{i}") for i in range(2)]
        havg = [pm.tile([128, BSH], F32, tag=f"havg_{i}", name=f"havg_{i}") for i in range(2)]

        def tgroup(Tt, off, g):
            nd = 8 if g < 7 else 4
            tp = pp.tile([128, 512], F32, tag="mm", bufs=2, name="tp")
            for d in range(nd):
                D = 8 * g + d
                nc.tensor.transpose(
                    tp[:, 64 * d : 64 * d + 64],
                    X[:, 64 * D + off : 64 * D + off + 128], ident[:],
                )
            nc.scalar.copy(
                Tt[:, 512 * g : 512 * g + 64 * nd], tp[:, : 64 * nd]
            )

        ones_col = pw.tile([128, 1], F32)
        nc.gpsimd.memset(ones_col[:], 1.0)
        ones_row = pw.tile([1, 128], F32)
        nc.gpsimd.memset(ones_row[:], 1.0)
        eps_col = pw.tile([1, 1], F32)
        nc.gpsimd.memset(eps_col[:], EPS)

        # remaining weights (issued after X so they don't delay transposes)
        w2t = pw.tile([128, 11 * 128], F32R)
        nc.sync.dma_start(w2t[:], w2t_d)
        w3t = pw.tile([128, 18 * 128], F32R)
        nc.sync.dma_start(w3t[:], w3t_d)
        w_inT = pw.tile([128, 2 * 1160], F32)
        nc.sync.dma_start(w_inT[:], w_inT_d)
        w_outT = pw.tile([128, 1024], F32)
        nc.sync.dma_start(w_outT[:], w_outT_d)
        f1wT = pw.tile([128, 128], F32)
        nc.sync.dma_start(f1wT[:], f1wT_d)
        f2wT = pw.tile([64, 1], F32)
        nc.sync.dma_start(f2wT[:], f2wT_d)
        emat = pw.tile([8, 512], F32)
        nc.sync.dma_start(emat[:], emat_d)
        # ---- BN scale/bias precompute: s = g/sqrt(v+eps); c = (b0-m)*s+beta
        s_all = pw.tile([128, 5], F32)
        c_all = pw.tile([128, 5], F32)
        tmpv = pw.tile([128, 5], F32)
        nc.vector.tensor_scalar_add(tmpv[:], vecs[:, 15:20], EPS)
        nc.scalar.sqrt(tmpv[:], tmpv[:])
        nc.vector.reciprocal(tmpv[:], tmpv[:])
        nc.vector.tensor_mul(s_all[:], vecs[:, 0:5], tmpv[:])
        nc.vector.tensor_sub(tmpv[:], vecs[:, 20:25], vecs[:, 10:15])
        nc.vector.tensor_mul(tmpv[:], tmpv[:], s_all[:])
        nc.vector.tensor_add(c_all[:], tmpv[:], vecs[:, 5:10])

        # ---- conv1 + maxpool(4) + bn + relu (interleaved with transposes) ----
        # out w = 8C + j + 4*delta; C = 2D (+1 odd); rhs cols (D, bh)
        p1v = P1[:].rearrange("p (c b) -> p c b", b=64)

        def conv1_chunk(n):
            cs = slice(256 * n, 256 * n + 256)
            for par in range(2):
                Tt = T if par == 0 else T2
                idx = (2 * n + par) % 3
                if idx < 2:
                    ps = pp.tile([128, 1024], F32, tag="c1", bufs=2, name="c1")
                else:
                    ps = pp.tile([128, 1024], F32, tag="acc", bufs=1, name="c1a")
                for j in range(4):
                    nc.tensor.matmul(
                        ps[:, 256 * j : 256 * j + 256],
                        w1t[:, 128 * j : 128 * j + 128],
                        Tt[0:79, cs], start=True, stop=True,
                    )
                nc.vector.tensor_reduce(
                    p1v[:, 8 * n + 5 + par : 8 * n + 13 + par : 2, :],
                    ps[:].rearrange("p (j x) -> p x j", j=4),
                    AX.X, ALU.max,
                )
            nc.scalar.activation(
                P1[:, (8 * n + 5) * 64 : (8 * n + 5) * 64 + 512],
                P1[:, (8 * n + 5) * 64 : (8 * n + 5) * 64 + 512],
                AF.Relu, bias=c_all[:, 0:1], scale=s_all[:, 0:1],
            )

        def conv2_chunk(n):
            ps = pp.tile([128, 512], F32, tag="mm", bufs=2, name="c2")
            for jp in range(11):
                nc.tensor.matmul(
                    ps[:],
                    w2t[:, 128 * jp : 128 * jp + 128],
                    P1[:, (8 * n + jp) * 64 : (8 * n + jp) * 64 + 512],
                    start=(jp == 0), stop=(jp == 10),
                )
            nc.scalar.activation(
                C3in[:, 256 + 512 * n : 256 + 512 * n + 512], ps[:],
                AF.Relu, bias=c_all[:, 1:2], scale=s_all[:, 1:2],
            )

        c3v = C3in[:].rearrange("p (w b) -> p w b", b=64)
        chunks3 = [(8 * i, 8) for i in range(7)] + [(56, 4)]

        def conv3_chunk(hf, ci):
            v0, nv = chunks3[ci]
            ps = pp.tile([128, 512], F32, tag="mm", bufs=2, name="c3")
            out_ap = ps[:, : nv * 64]
            for k in range(9):
                rhs = c3v[:, 2 * v0 + k : 2 * v0 + k + 2 * nv : 2, :]
                nc.tensor.matmul(
                    ps[:, : nv * 64],
                    w3t[:, (hf * 9 + k) * 128 : (hf * 9 + k) * 128 + 128],
                    rhs,
                    start=(k == 0), stop=(k == 8),
                )
            nc.scalar.activation(
                H3[hf][:, 64 * v0 : 64 * (v0 + nv)], out_ap,
                AF.Relu, bias=c_all[:, 2 + hf : 3 + hf],
                scale=s_all[:, 2 + hf : 3 + hf],
            )
            hv = H3[hf][:, 64 * v0 : 64 * (v0 + nv)].rearrange(
                "p (v b h) -> p b v h", v=nv, b=32, h=2
            )
            if ci == 0:
                nc.vector.tensor_reduce(havg[hf][:], hv, AX.XY, ALU.add)
            else:
                hp = pt.tile([128, BSH], F32, tag="hp", name="hp")
                nc.vector.tensor_reduce(hp[:], hv, AX.XY, ALU.add)
                nc.vector.tensor_add(havg[hf][:], havg[hf][:], hp[:])
            if ci == len(chunks3) - 1:
                nc.vector.tensor_scalar_mul(havg[hf][:], havg[hf][:], 1.0 / 120.0)

        # interleaved emission: conv1(n) -> conv2(n-3) -> conv3(hf0, ...)
        state = {"e1": 0, "e2": 0, "e3": 0}

        def pump():
            while state["e2"] <= state["e1"] - 3 and state["e2"] < 15:
                conv2_chunk(state["e2"])
                state["e2"] += 1
                while state["e3"] < 8 and 2 * state["e3"] + 3 <= state["e2"] - 1:
                    conv3_chunk(0, state["e3"])
                    state["e3"] += 1

        for g in range(8):
            tgroup(T, 0, g)
            tgroup(T2, 32, g)
            while state["e1"] <= 2 * g - 1 and state["e1"] < 15:
                conv1_chunk(state["e1"])
                state["e1"] += 1
                pump()
        while state["e1"] < 15:
            conv1_chunk(state["e1"])
            state["e1"] += 1
            pump()
        while state["e2"] < 15:
            conv2_chunk(state["e2"])
            state["e2"] += 1
            while state["e3"] < 8 and 2 * state["e3"] + 3 <= state["e2"] - 1:
                conv3_chunk(0, state["e3"])
                state["e3"] += 1
        while state["e3"] < 8:
            conv3_chunk(0, state["e3"])
            state["e3"] += 1

        for ci in range(8):
            conv3_chunk(1, ci)

        # in_proj: M-tiles (z:0-3, xBC, dt), K=2x128
        ip = pp.tile([128, 352], F32, tag="c1", bufs=2, name="ip")
        mtiles = [(10, 1152, 8), (8, 1024, 64), (9, 1088, 64)]
        mtiles += [(m, 128 * m, 128) for m in range(4, 8)]
        mtiles += [(m, 128 * m, 128) for m in range(4)]
        for m, f0, mm in mtiles:
            for k in range(2):
                nc.tensor.matmul(
                    ip[0:mm, 32 * m : 32 * m + 32],
                    w_inT[:, 1160 * k + f0 : 1160 * k + f0 + mm],
                    havg[k][:],
                    start=(k == 0), stop=(k == 1),
                )

        # ---- mamba + classifier (feature-major, batch on free dim) ----
        xcB = pt.tile([64, BSH], F32, tag="xcB")
        nc.scalar.activation(
            xcB[:], ip[0:64, 256:288], AF.Silu,
            bias=vecs[0:64, 37:38], scale=vecs[0:64, 32:33],
        )
        xcC = pt.tile([64, BSH], F32, tag="xcC")
        nc.scalar.activation(
            xcC[:], ip[0:64, 288:320], AF.Silu,
            bias=vecs[0:64, 43:44], scale=vecs[0:64, 42:43],
        )
        dts = pt.tile([8, BSH], F32, tag="dts")
        # softplus(x + b) = ln(1 + exp(x + b)) (no softplus ACT table here)
        nc.scalar.activation(
            dts[:], ip[0:8, 320:352], AF.Exp, bias=vecs[0:8, 25:26]
        )
        nc.scalar.activation(dts[:], dts[:], AF.Ln, bias=1.0)
        xc = [pt.tile([128, BSH], F32, tag=f"xc{m}", name=f"xc{m}") for m in range(4)]
        for m in range(4):
            nc.scalar.activation(
                xc[m][:], ip[:, 32 * (4 + m) : 32 * (4 + m) + 32], AF.Silu,
                bias=vecs[:, 33 + m : 34 + m], scale=vecs[:, 28 + m : 29 + m],
            )
        zsall = pt.tile([128, 4 * BSH], F32, tag="zsall")
        nc.scalar.activation(zsall[:], ip[:, 0:128], AF.Silu)
        zs = [zsall[:, 32 * m : 32 * m + 32] for m in range(4)]

        # s = sum_f Bm*Cm  (per batch scalar), via ones-matmul
        bc = pt.tile([64, BSH], F32, tag="bc")
        nc.vector.tensor_mul(bc[:], xcB[:], xcC[:])
        ps_s = pp.tile([1, BSH], F32, tag="mm", bufs=2, name="ps_s")
        nc.tensor.matmul(ps_s[:], ones_col[0:64, :], bc[:], start=True, stop=True)
        s_sb = pt.tile([1, BSH], F32, tag="s_sb")
        nc.vector.tensor_copy(s_sb[:], ps_s[:])
        ps_s8 = pp.tile([8, BSH], F32, tag="mm", bufs=2, name="ps_s8")
        nc.tensor.matmul(ps_s8[:], ones_row[0:1, 0:8], s_sb[:], start=True, stop=True)
        g = pt.tile([8, BSH], F32, tag="g")
        nc.vector.tensor_mul(g[:], dts[:], ps_s8[:])
        nc.vector.tensor_scalar_add(g[:], g[:], vecs[0:8, 26:27])

        y = [pt.tile([128, BSH], F32, tag=f"y{t}", name=f"y{t}") for t in range(4)]
        ps_ms = pp.tile([1, BSH], F32, tag="c1", bufs=2, name="ps_ms")
        for t in range(4):
            ge = pp.tile([128, BSH], F32, tag="mm", bufs=2, name="ge")
            nc.tensor.matmul(ge[:], emat[:, 128 * t : 128 * t + 128], g[:],
                             start=True, stop=True)
            nc.vector.tensor_mul(y[t][:], xc[t][:], ge[:])
            nc.vector.tensor_mul(y[t][:], y[t][:], zs[t])
            sq = pt.tile([128, BSH], F32, tag="sq")
            nc.vector.tensor_mul(sq[:], y[t][:], y[t][:])
            nc.tensor.matmul(ps_ms[:], ones_col[:], sq[:],
                             start=(t == 0), stop=(t == 3))
        sd = pt.tile([1, BSH], F32, tag="sd")
        nc.scalar.activation(sd[:], ps_ms[:], AF.Sqrt,
                             bias=eps_col[:], scale=1.0 / 512.0)
        rinv = pt.tile([1, BSH], F32, tag="rinv")
        nc.vector.reciprocal(rinv[:], sd[:])
        ps_rb = pp.tile([128, BSH], F32, tag="mm", bufs=2, name="ps_rb")
        nc.tensor.matmul(ps_rb[:], ones_row[:], rinv[:], start=True, stop=True)

        yn = [pt.tile([128, BSH], F32, tag=f"yn{t}", name=f"yn{t}") for t in range(4)]
        for t in range(4):
            nc.vector.tensor_mul(yn[t][:], y[t][:], ps_rb[:])
            nc.vector.tensor_scalar_mul(yn[t][:], yn[t][:],
                                        vecs[:, 38 + t : 39 + t])

        # out_proj [256,512] @ yn -> o [256, 32] (2 M-tiles in one psum)
        ps_o = pp.tile([128, 64], F32, tag="mm", bufs=2, name="ps_o")
        for m in range(2):
            for k in range(4):
                nc.tensor.matmul(
                    ps_o[:, 32 * m : 32 * m + 32],
                    w_outT[:, (k * 2 + m) * 128 : (k * 2 + m) * 128 + 128],
                    yn[k][:],
                    start=(k == 0), stop=(k == 3),
                )
        o_sb = pt.tile([128, 64], F32, tag="o_sb")
        nc.vector.tensor_copy(o_sb[:], ps_o[:])

        # fc1 + bn4 + relu
        ps_f1 = pp.tile([64, BSH], F32, tag="c1", bufs=2, name="ps_f1")
        for k in range(2):
            nc.tensor.matmul(
                ps_f1[:], f1wT[:, 64 * k : 64 * k + 64],
                o_sb[:, 32 * k : 32 * k + 32],
                start=(k == 0), stop=(k == 1),
            )
        o1 = pt.tile([64, BSH], F32, tag="o1")
        nc.scalar.activation(o1[:], ps_f1[:], AF.Relu,
                             bias=c_all[0:64, 4:5], scale=s_all[0:64, 4:5])

        # fc2
        ps_f2 = pp.tile([1, BSH], F32, tag="c1", bufs=2, name="ps_f2")
        nc.tensor.matmul(ps_f2[:], f2wT[:], o1[:], start=True, stop=True)
        ores = pt.tile([1, BSH], F32, tag="ores")
        nc.scalar.activation(ores[:], ps_f2[:], AF.Identity,
                             bias=vecs[0:1, 27:28])
        nc.sync.dma_start(y_d, ores[:])


_NC_CACHE = []


def kernel(**inputs):
    if not _NC_CACHE:
        _NC_CACHE.append(_build_nc())
    nc = _NC_CACHE[0]
    w = _prep_weights(inputs)
    x = np.asarray(inputs["x"], np.float32)
    in_maps = []
    for c in range(NCORES):
        m = dict(w)
        m["x"] = np.ascontiguousarray(x[c * BSH : (c + 1) * BSH])
        in_maps.append(m)
    res = run_bass_kernel_spmd(nc, in_maps, list(range(NCORES))).results
    out = np.concatenate([res[c]["y"].reshape(BSH, 1) for c in range(NCORES)], 0)
    return out



# revision 18
# speedup vs baseline: 1.2965x; 1.2965x over previous
"""Trainium2 Bass kernel for nn_CNN_MAMBA2 (CNN + Mamba2(L=1) + MLP head).

Strategy: pure data parallel over batch (B=256 -> 32 per core x 8 cores).
Each core runs the full network on its batch shard; weights are replicated.

This version keeps the conv math on the PE array in fp8 (e4m3) DoubleRow
perf mode (2 K-tiles of 128 per instruction at 0.5 cycles/row), which is
4x the fp32r/bf16 row rate:

  X8   [64, 3968]  fp8(8*x), batch-major, padded (xpad[i] = x[i-25])
  T8a/b [128, 960] u16-pair DMA transposes of X8: partition p of block f
        holds positions (256f + 2p, 256f + 2p + 1) as two fp8 bytes; the
        byte index is the DoubleRow K-tile for conv1.  T8a covers window
        phases E%4 in {0,2} (partition base 0/64), T8b = shifted 64
        positions for odd phases.
  conv1: window E serves out w in [16E, 16E+15]; one DR matmul per
        (j, dgroup) with M = (64ch x 2 pool-parity), tap offsets packed
        into (q, byte) of the 128-position window.  PSUM regions (dg, j).
  maxpool(4) + bn1 + relu fused into a DVE/Pool cascade:
        stage1 = pairwise max (j0,j1)/(j2,j3) -> bf16, stage2 = STT
        max(max(.,thresh),.) -> fp8 P1 (thresh implements relu for
        folded bn bias; bn scale s1 is folded into w2 rows on device).
  P1   [128, 8320] fp8, partition = 64*delta + ci, col = (C+5)*64 + bh
  conv2: 21 taps -> 12 K-tiles -> 6 DR matmuls, plus 6 more DR with the
        same-scale fp8 residual of the weights (double-fp8 weights kill
        the systematic weight-quantization bias).  bn2+relu+requant in
        the ACT evacuation (per-partition scale/bias APs).
  C3in [128, 8192] fp8, col = (u+4)*64 + bh
  conv3: 4 DR (taps 0-7) + 1 single fp8 matmul (tap 8) per (hf, chunk);
        ACT evacuation -> fp32 tmp; avgpool = per-chunk tensor_reduce
        over v (DVE/Pool alternating) accumulated into hsum.
  Mamba2 with L=1 and the classifier run in fp32 exactly as the
        reference (feature-major, batch on the free dim).

Host-side prep is layout-only (transpose/reshape/pad/tile of weights); all
arithmetic (scaling, fp8 quantization, BN folding, silu, conv, matmuls,
norms) happens on device.
"""

import numpy as np

import bass_rust
import concourse.bass as bass
import concourse.mybir as mybir
from concourse.tile import TileContext
from concourse.bass_utils import run_bass_kernel_spmd

F32 = mybir.dt.float32
BF16 = mybir.dt.bfloat16
F8 = mybir.dt.float8e4
U16 = mybir.dt.uint16
AF = mybir.ActivationFunctionType
ALU = mybir.AluOpType
AX = mybir.AxisListType
PM = mybir.MatmulPerfMode

EPS = 1e-5
NCORES = 8
BSH = 32            # batches per core
BH = 64             # bh signals per core (32 b x 2 h rows)
XPAD = 3968         # padded positions, 31*128 for u16 transpose alignment

SX = 8.0            # fp8 scale for x
SW1 = 16.0          # fp8 scale for conv1 weights
SP1 = SX * SW1      # P1 scale (128)
SW2 = 32.0          # conv2 weight scale (on top of folded s1)
SC3 = 64.0          # C3in fp8 scale
SW3 = 32.0          # conv3 weight scale


def _split_multi_waits(nc):
    """This walrus build accepts at most one sync-wait command per
    instruction; Tile's sem assignment attaches several. Hoist extra waits
    onto dedicated single-wait nops right before the instruction (same
    engine), which preserves blocking semantics."""
    n = 0
    for fn in nc.m.functions:
        for bb in fn.blocks:
            out = []
            for inst in bb.instructions:
                si = inst.sync_info
                waits = list(si.on_wait) if si is not None else []
                if len(waits) > 1:
                    for w in waits[:-1]:
                        n += 1
                        nop = mybir.InstNoOp(name=f"waitnop-{n}", ins=[], outs=[])
                        nop.engine = inst.engine
                        nop.debug = inst.debug
                        nop.sync_info = bass_rust.SyncInfo(
                            on_wait=[w], on_update=[]
                        )
                        out.append(nop)
                    si.on_wait = [waits[-1]]
                    inst.sync_info = si
                out.append(inst)
            bb.instructions = out


# --------------------------------------------------------------------------
# host-side weight layout prep (layout only: transpose / reshape / pad / tile)
# --------------------------------------------------------------------------

def _prep_weights(inp):
    f32 = np.float32
    c1w = np.asarray(inp["c1w"], f32).reshape(64, 51)
    # conv1 DR layout: [q, (j, dg) 8, kt 2, m 128]; rows duplicated to both
    # partition halves so lhsT base can match the rhs window base.
    w1dr = np.zeros((64, 8, 2, 128), f32)
    for j in range(4):
        for dp in range(4):               # delta' = pool block within window
            for k in range(51):
                o = 4 * j + 16 * dp + k
                m = 64 * (dp & 1)
                w1dr[o >> 1, 4 * (dp >> 1) + j, o & 1, m:m + 64] = c1w[:, k]
    w1dr = w1dr.reshape(64, 2048)
    w1dr = np.concatenate([w1dr, w1dr], axis=0)          # [128, 2048]

    c2w = np.asarray(inp["c2w"], f32).reshape(128, 64, 21)
    # conv2 DR layout: [64d+ci, rho 6, kt 2, oc 128]
    w2dr = np.zeros((128, 6, 2, 128), f32)
    for rho in range(6):
        for kt in range(2):
            for d in range(2):
                t = 2 * (2 * rho + kt) + d
                if t <= 20:
                    w2dr[64 * d:64 * d + 64, rho, kt, :] = c2w[:, :, t].T
    w2dr = w2dr.reshape(128, 1536)

    c3w = np.asarray(inp["c3w"], f32).reshape(256, 128, 9)
    # conv3: DR taps 0..7 [ci, hf 2, rho 4, kt 2, oc 128] + single tap 8
    w3dr = np.zeros((128, 2, 4, 2, 128), f32)
    w3s = np.zeros((128, 2, 128), f32)
    for hf in range(2):
        for rho in range(4):
            for kt in range(2):
                w3dr[:, hf, rho, kt, :] = c3w[128 * hf:128 * hf + 128, :, 2 * rho + kt].T
        w3s[:, hf, :] = c3w[128 * hf:128 * hf + 128, :, 8].T
    w3all = np.concatenate([w3dr.reshape(128, 2048), w3s.reshape(128, 256)], 1)

    mw_in = np.asarray(inp["mw_in"], f32)          # [1160, 256]
    w_inT = np.zeros((128, 2, 1160), f32)
    for k in range(2):
        w_inT[:, k, :] = mw_in[:, 128 * k:128 * k + 128].T

    mw_out = np.asarray(inp["mw_out"], f32)        # [256, 512]
    w_outT = np.zeros((128, 4, 2, 128), f32)
    for k in range(4):
        for m in range(2):
            w_outT[:, k, m, :] = mw_out[
                128 * m:128 * m + 128, 128 * k:128 * k + 128
            ].T

    f1w = np.asarray(inp["f1w"], f32)              # [64, 256]
    f1wT = np.zeros((128, 2, 64), f32)
    for k in range(2):
        f1wT[:, k, :] = f1w[:, 128 * k:128 * k + 128].T

    f2wT = np.asarray(inp["f2w"], f32).reshape(1, 64).T.copy()   # [64, 1]

    def t2(a):
        return np.tile(np.asarray(a, f32), 2)

    def pd(a):
        a = np.asarray(a, f32)
        return np.pad(a, (0, 128 - a.shape[0]))

    vecs = np.zeros((128, 44), f32)
    # cols 0-4 bn gammas, 5-9 betas, 10-14 means, 15-19 vars, 20-24 pre-bias
    vecs[:, 0] = t2(inp["bn1g"]); vecs[:, 5] = t2(inp["bn1b"])
    vecs[:, 10] = t2(inp["bn1m"]); vecs[:, 15] = t2(inp["bn1v"])
    vecs[:, 20] = t2(inp["c1b"])
    vecs[:, 1] = inp["bn2g"]; vecs[:, 6] = inp["bn2b"]
    vecs[:, 11] = inp["bn2m"]; vecs[:, 16] = inp["bn2v"]
    vecs[:, 21] = inp["c2b"]
    for hf in range(2):
        s = slice(128 * hf, 128 * hf + 128)
        vecs[:, 2 + hf] = inp["bn3g"][s]; vecs[:, 7 + hf] = inp["bn3b"][s]
        vecs[:, 12 + hf] = inp["bn3m"][s]; vecs[:, 17 + hf] = inp["bn3v"][s]
        vecs[:, 22 + hf] = inp["c3b"][s]
    vecs[:, 4] = pd(inp["bn4g"]); vecs[:, 9] = pd(inp["bn4b"])
    vecs[:, 14] = pd(inp["bn4m"]); vecs[:, 19] = pd(inp["bn4v"])
    vecs[:, 24] = pd(inp["f1b"])
    vecs[0:8, 25] = inp["mdt_bias"]
    vecs[0:8, 26] = inp["mD"]
    vecs[0:1, 27] = inp["f2b"]
    mcw = np.asarray(inp["mconv_w"], f32)[:, 0, 3]
    mcb = np.asarray(inp["mconv_b"], f32)
    vecs[:, 28:33] = mcw.reshape(5, 128).T
    vecs[:, 33:38] = mcb.reshape(5, 128).T
    vecs[:, 38:42] = np.asarray(inp["mnorm_w"], f32).reshape(4, 128).T
    vecs[0:64, 42] = mcw[576:640]
    vecs[0:64, 43] = mcb[576:640]

    # constant head-expansion matrix: emat[h, 128*t + m] = 1 iff h == 2t + m//64
    emat = np.zeros((8, 512), np.float32)
    for t in range(4):
        emat[2 * t, 128 * t:128 * t + 64] = 1.0
        emat[2 * t + 1, 128 * t + 64:128 * t + 128] = 1.0

    return {
        "w1dr": w1dr, "w2dr": w2dr, "w3all": w3all,
        "w_inT": w_inT.reshape(128, -1), "w_outT": w_outT.reshape(128, -1),
        "f1wT": f1wT.reshape(128, -1), "f2wT": f2wT, "vecs": vecs, "emat": emat,
    }


# --------------------------------------------------------------------------
# device kernel
# --------------------------------------------------------------------------

def _dr_rhs(ap, nparts, dims):
    """Build a DoubleRow rhs AP with custom free dims (list of [stride, n]);
    `ap` must be a 2D slice spanning at least the same byte range."""
    v = ap.unsqueeze(1).broadcast_to([nparts, 2, ap.shape[-1]])
    cur = [list(p) for p in v.ap]
    v.ap = bass_rust.VecI64Pair([cur[0]] + dims)
    return v


def _build_nc():
    nc = bass.Bass("TRN2", target_bir_lowering=False, debug=False)

    x_d = nc.dram_tensor("x", [BSH, 2, 3840], F32, kind="ExternalInput").ap()
    w1dr_d = nc.dram_tensor("w1dr", [128, 2048], F32, kind="ExternalInput").ap()
    w2dr_d = nc.dram_tensor("w2dr", [128, 1536], F32, kind="ExternalInput").ap()
    w3all_d = nc.dram_tensor("w3all", [128, 2304], F32, kind="ExternalInput").ap()
    w_inT_d = nc.dram_tensor("w_inT", [128, 2 * 1160], F32, kind="ExternalInput").ap()
    w_outT_d = nc.dram_tensor("w_outT", [128, 1024], F32, kind="ExternalInput").ap()
    f1wT_d = nc.dram_tensor("f1wT", [128, 128], F32, kind="ExternalInput").ap()
    f2wT_d = nc.dram_tensor("f2wT", [64, 1], F32, kind="ExternalInput").ap()
    vecs_d = nc.dram_tensor("vecs", [128, 44], F32, kind="ExternalInput").ap()
    emat_d = nc.dram_tensor("emat", [8, 512], F32, kind="ExternalInput").ap()
    y_d = nc.dram_tensor("y", [1, BSH], F32, kind="ExternalOutput").ap()

    with TileContext(nc) as tc:
        _body(nc, tc, x_d, w1dr_d, w2dr_d, w3all_d, w_inT_d, w_outT_d,
              f1wT_d, f2wT_d, vecs_d, emat_d, y_d)
    _split_multi_waits(nc)
    return nc


def _body(nc, tc, x_d, w1dr_d, w2dr_d, w3all_d, w_inT_d, w_outT_d,
          f1wT_d, f2wT_d, vecs_d, emat_d, y_d):
    with (
        tc.tile_pool(name="pw", bufs=1) as pw,
        tc.tile_pool(name="pmain", bufs=1) as pm,
        tc.tile_pool(name="ptmp", bufs=3) as pt,
        tc.tile_pool(name="pp", bufs=1, space="PSUM") as pp,
    ):
        # ---- x load (3 chunks) + fp8 conversion ----
        X = pm.tile([64, 3840], F32)          # data cols only (positions 25..3864)
        xflat = x_d.rearrange("b h w -> (b h) w")
        X8 = pm.tile([64, XPAD], F8)
        nc.gpsimd.memset(X8[:, 0:25], 0.0)
        nc.gpsimd.memset(X8[:, 3865:XPAD], 0.0)
        for c in range(3):
            nc.sync.dma_start(X[:, 1280 * c:1280 * c + 1280],
                              xflat[:, 1280 * c:1280 * c + 1280])
        # conversion chunks (positions 25..3865 = X cols 0..3840)
        ccuts = [25, 1280, 2560, 3865]
        for c in range(3):
            p0, p1 = ccuts[c], ccuts[c + 1]
            eng = nc.scalar if c < 2 else nc.gpsimd
            if eng is nc.scalar:
                nc.scalar.activation(X8[:, p0:p1], X[:, p0 - 25:p1 - 25],
                                     AF.Copy, scale=SX)
            else:
                nc.gpsimd.tensor_scalar_mul(X8[:, p0:p1], X[:, p0 - 25:p1 - 25], SX)

        w1dr = pw.tile([128, 2048], F32)
        nc.sync.dma_start(w1dr[:], w1dr_d)
        vecs = pw.tile([128, 44], F32)
        nc.sync.dma_start(vecs[:], vecs_d)

        # ---- T8a / T8b: u16-pair DMA transposes, 3 chunks each ----
        T8a = pm.tile([128, 1920], F8, name="t8a")
        T8b = pm.tile([128, 1920], F8, name="t8b")
        X8u = X8[:].bitcast(U16)              # [64, 1984]
        T8au = T8a[:].bitcast(U16)            # [128, 960]
        T8bu = T8b[:].bitcast(U16)
        for c in range(3):
            nc.sync.dma_start_transpose(
                T8au[:, 320 * c:320 * c + 320].rearrange(
                    "p (f b) -> p f b", f=5, b=64),
                X8u[:, 640 * c:640 * c + 640],
            )
            nc.sync.dma_start_transpose(
                T8bu[:, 320 * c:320 * c + 320].rearrange(
                    "p (f b) -> p f b", f=5, b=64),
                X8u[:, 640 * c + 32:640 * c + 672],
            )

        # ---- fp8 weight conversion on device ----
        w1q = pw.tile([128, 2048], F8)
        nc.scalar.activation(w1q[:], w1dr[:], AF.Copy, scale=SW1)

        # bn scale/bias precompute: s = g/sqrt(v+eps); c = (b0-m)*s+beta
        s_all = pw.tile([128, 5], F32)
        c_all = pw.tile([128, 5], F32)
        tmpv = pw.tile([128, 5], F32)
        nc.vector.tensor_scalar_add(tmpv[:], vecs[:, 15:20], EPS)
        nc.scalar.sqrt(tmpv[:], tmpv[:])
        nc.vector.reciprocal(tmpv[:], tmpv[:])
        nc.vector.tensor_mul(s_all[:], vecs[:, 0:5], tmpv[:])
        nc.vector.tensor_sub(tmpv[:], vecs[:, 20:25], vecs[:, 10:15])
        nc.vector.tensor_mul(tmpv[:], tmpv[:], s_all[:])
        nc.vector.tensor_add(c_all[:], tmpv[:], vecs[:, 5:10])

        # conv1 relu threshold thresh = -SP1 * c1eff / s1  (per P1 partition)
        smallv = pw.tile([128, 8], F32)
        # col0: 1/s1; col1: thresh; col2: s1*SW2; col3: scale2 = s2*SC3/(SW2*SP1)
        # col4: bias2; col5: scale3 lo half; col6: scale3 hi; col7: tmp
        nc.vector.reciprocal(smallv[:, 0:1], s_all[:, 0:1])
        nc.vector.tensor_mul(smallv[:, 1:2], c_all[:, 0:1], smallv[:, 0:1])
        nc.vector.tensor_scalar_mul(smallv[:, 1:2], smallv[:, 1:2], -SP1)
        nc.vector.tensor_scalar_mul(smallv[:, 2:3], s_all[:, 0:1], SW2)
        nc.vector.tensor_scalar_mul(smallv[:, 3:4], s_all[:, 1:2],
                                    SC3 / (SW2 * SP1))
        nc.vector.tensor_scalar_mul(smallv[:, 5:6], s_all[:, 2:3],
                                    1.0 / (SC3 * SW3))
        nc.vector.tensor_scalar_mul(smallv[:, 6:7], s_all[:, 3:4],
                                    1.0 / (SC3 * SW3))
        thresh = smallv[:, 1:2]
        thresh8 = pw.tile([128, 1], F8)
        nc.scalar.activation(thresh8[:], thresh, AF.Copy, scale=1.0)

        # conv2 weights: hi = fp8(w2 * s1 * SW2), lo = fp8 residual (same scale)
        w2dr = pw.tile([128, 1536], F32)
        nc.sync.dma_start(w2dr[:], w2dr_d)
        w2hi = pw.tile([128, 1536], F8)
        nc.scalar.activation(w2hi[:], w2dr[:], AF.Copy, scale=smallv[:, 2:3])
        w2lo = pw.tile([128, 1536], F8)
        nc.vector.scalar_tensor_tensor(w2lo[:], w2dr[:], smallv[:, 2:3],
                                       w2hi[:], op0=ALU.mult, op1=ALU.subtract)

        # conv3 weights
        w3all = pw.tile([128, 2304], F32)
        nc.sync.dma_start(w3all[:], w3all_d)
        w3q = pw.tile([128, 2304], F8)
        nc.scalar.activation(w3q[:], w3all[:], AF.Copy, scale=SW3)

        # comp[oc] = sum_kappa w2q[kappa, oc] * thresh[kappa]  (threshold offset
        # compensation; exactly zero for zero conv/bn biases)
        ps_cmp = pp.tile([128, 8], F32, tag="mm", bufs=2, name="ps_cmp")
        for i in range(12):
            for wsrc in (w2hi, w2lo):
                nc.tensor.matmul(
                    ps_cmp[:, 0:1], wsrc[:, 128 * i:128 * i + 128], thresh8[:],
                    start=(i == 0 and wsrc is w2hi),
                    stop=(i == 11 and wsrc is w2lo),
                )
        comp = pw.tile([128, 1], F32)
        nc.vector.tensor_copy(comp[:], ps_cmp[:, 0:1])
        # bias2 = SC3*c2eff - scale2*comp
        bias2 = pw.tile([128, 1], F32)
        nc.vector.tensor_mul(bias2[:], smallv[:, 3:4], comp[:])
        nc.vector.scalar_tensor_tensor(bias2[:], c_all[:, 1:2], SC3, bias2[:],
                                       op0=ALU.mult, op1=ALU.subtract)

        # remaining fp32 weights (used late)
        w_outT = pw.tile([128, 1024], F32)
        nc.sync.dma_start(w_outT[:], w_outT_d)
        f1wT = pw.tile([128, 128], F32)
        nc.sync.dma_start(f1wT[:], f1wT_d)
        f2wT = pw.tile([64, 1], F32)
        nc.sync.dma_start(f2wT[:], f2wT_d)
        emat = pw.tile([8, 512], F32)
        nc.sync.dma_start(emat[:], emat_d)
        w_inT = pw.tile([128, 2 * 1160], F32)
        nc.sync.dma_start(w_inT[:], w_inT_d)

        ones_col = pw.tile([128, 1], F32)
        nc.gpsimd.memset(ones_col[:], 1.0)
        ones_row = pw.tile([1, 128], F32)
        nc.gpsimd.memset(ones_row[:], 1.0)
        ones8 = pw.tile([64, 8], F32)
        nc.gpsimd.memset(ones8[:], 1.0)
        eps_col = pw.tile([1, 1], F32)
        nc.gpsimd.memset(eps_col[:], EPS)

        # ---- main tiles ----
        P1 = pm.tile([128, 8384], F8)
        nc.gpsimd.memset(P1[:, 0:320], 0.0)
        nc.gpsimd.memset(P1[:, 8000:8384], 0.0)
        # C3in parity-split layout: col = (u%2)*4096 + ((u+4)//2)*64 + b, so
        # conv3's tap pairs (even tap, odd tap) read the same block index in
        # the two 4096-halves (kt stride 4096) with (v, b) contiguous.
        C3in = pm.tile([128, 8192], F8)
        nc.gpsimd.memset(C3in[:, 0:128], 0.0)
        nc.gpsimd.memset(C3in[:, 3968:4224], 0.0)
        nc.gpsimd.memset(C3in[:, 8064:8192], 0.0)
        hsum = [pm.tile([128, 64], F32, tag=f"hsum{i}", name=f"hsum{i}")
                for i in range(2)]
        havg = [pm.tile([128, BSH], F32, tag=f"havg{i}", name=f"havg{i}")
                for i in range(2)]

        # ---- conv1 chunk: group g in 0..3 (T8, partbase), c in 0..4 ----
        groups = [(T8a, 0, 0), (T8b, 0, 1), (T8a, 64, 2), (T8b, 64, 3)]

        def conv1_chunk(g, c):
            # chunk c covers windows f = 2c, 2c+1 (c < 7), f = 14 for c == 7
            T8, pb, g0 = groups[g]
            nf = 2 if c < 7 else 1
            nn = 64 * nf
            ps = pp.tile([128, 1024], F32, tag="c1", bufs=2, name="c1ps")
            rhs = _dr_rhs(T8[pb:pb + 64, 256 * c:256 * c + 128 * nf], 64,
                          [[1, 2], [2, nn]])
            for dg in range(2):
                for j in range(4):
                    lhs = w1q[pb:pb + 64,
                              (4 * dg + j) * 256:(4 * dg + j) * 256 + 256]
                    lhs = lhs.rearrange("p (kt m) -> p kt m", kt=2)
                    nc.tensor.matmul(
                        ps[:, (4 * dg + j) * 128:(4 * dg + j) * 128 + nn],
                        lhs, rhs, start=True, stop=True, perf_mode=PM.DoubleRow,
                    )
            # maxpool cascade; psum regions (dg, j) are 128-strided, nn valid
            psr = ps[:].rearrange("p (dg j w) -> p j dg w", dg=2, j=4, w=128)
            R1 = pt.tile([128, 512], BF16, tag="r1", name="r1")
            R1r = R1[:, 0:4 * nn].rearrange("p (jj dg w) -> p jj dg w",
                                            jj=2, dg=2, w=nn)
            if (g + c) % 2 == 0:
                nc.gpsimd.tensor_tensor(R1r, psr[:, 0:3:2, :, 0:nn],
                                        psr[:, 1:4:2, :, 0:nn], ALU.max)
            else:
                nc.vector.scalar_tensor_tensor(R1r, psr[:, 0:3:2, :, 0:nn],
                                               1.0, psr[:, 1:4:2, :, 0:nn],
                                               op0=ALU.mult, op1=ALU.max)
            # stage2 -> P1, col C = 2*g0 + 16c + 8fi + dg
            base = (2 * g0 + 16 * c + 5) * 64
            span = 64 + 512 * (nf - 1) + 64
            out = P1[:, base:base + span].unsqueeze(1).broadcast_to(
                [128, 2, span])
            cur = [list(p) for p in out.ap]
            out.ap = bass_rust.VecI64Pair(
                [cur[0], [64, 2], [512, nf], [1, 64]])
            nc.vector.scalar_tensor_tensor(
                out, R1[:, 0:2 * nn].rearrange("p (dg fi b) -> p dg fi b",
                                               dg=2, fi=nf, b=64),
                thresh,
                R1[:, 2 * nn:4 * nn].rearrange("p (dg fi b) -> p dg fi b",
                                               dg=2, fi=nf, b=64),
                op0=ALU.max, op1=ALU.max)

        # ---- conv2 chunk n in 0..14 ----
        def conv2_chunk(n):
            ps = pp.tile([128, 512], F32, tag="mm", bufs=2, name="c2ps")
            i = 0
            for wsrc in (w2hi, w2lo):
                for rho in range(6):
                    base = (8 * n + 2 * rho) * 64
                    rhs = _dr_rhs(P1[:, base:base + 576], 128,
                                  [[64, 2], [1, 512]])
                    lhs = wsrc[:, rho * 256:rho * 256 + 256].rearrange(
                        "p (kt m) -> p kt m", kt=2)
                    nc.tensor.matmul(ps[:], lhs, rhs,
                                     start=(i == 0), stop=(i == 11),
                                     perf_mode=PM.DoubleRow)
                    i += 1
            base = 256 * n + 128
            out = C3in[:, base:base + 4352].unsqueeze(1).broadcast_to(
                [128, 2, 4352])
            cur = [list(p) for p in out.ap]
            out.ap = bass_rust.VecI64Pair(
                [cur[0], [64, 4], [4096, 2], [1, 64]])
            nc.scalar.activation(
                out, ps[:].rearrange("p (q r b) -> p q r b", q=4, r=2, b=64),
                AF.Relu, bias=bias2[:], scale=smallv[:, 3:4],
            )

        # ---- conv3 chunk (hf, ci) ----
        chunks3 = [(8 * i, 8) for i in range(7)] + [(56, 4)]

        def conv3_chunk(hf, ci):
            v0, nv = chunks3[ci]
            ps = pp.tile([128, 512], F32, tag="mm", bufs=2, name="c3ps")
            for rho in range(4):
                lhs = w3q[:, (hf * 4 + rho) * 256:(hf * 4 + rho) * 256 + 256]
                lhs = lhs.rearrange("p (kt m) -> p kt m", kt=2)
                rhs = _dr_rhs(C3in[:, (v0 + rho) * 64:
                                    (v0 + rho) * 64 + 4096 + 64 * nv],
                              128, [[4096, 2], [1, 64 * nv]])
                nc.tensor.matmul(
                    ps[:, :nv * 64], lhs, rhs,
                    start=(rho == 0), stop=False, perf_mode=PM.DoubleRow)
            nc.tensor.matmul(
                ps[:, :nv * 64], w3q[:, 2048 + 128 * hf:2048 + 128 * hf + 128],
                C3in[:, (v0 + 4) * 64:(v0 + 4) * 64 + 64 * nv],
                start=False, stop=True)
            h3t = pt.tile([128, 512], F32, tag="h3t", name="h3t")
            nc.scalar.activation(
                h3t[:, :nv * 64], ps[:, :nv * 64], AF.Relu,
                bias=c_all[:, 2 + hf:3 + hf], scale=smallv[:, 5 + hf:6 + hf],
            )
            # avgpool partial: reduce over v -> [128, 64]
            hv = h3t[:, :nv * 64].rearrange("p (v b) -> p b v", v=nv, b=64)
            if ci == 0:
                nc.vector.tensor_reduce(hsum[hf][:], hv, AX.X, ALU.add)
            else:
                hp = pt.tile([128, 64], F32, tag="hp", name="hp")
                nc.vector.tensor_reduce(hp[:], hv, AX.X, ALU.add)
                nc.gpsimd.tensor_tensor(hsum[hf][:], hsum[hf][:], hp[:], ALU.add)
            if ci == len(chunks3) - 1:
                hs2 = hsum[hf][:].rearrange("p (b h) -> p b h", h=2)
                nc.vector.tensor_reduce(havg[hf][:], hs2, AX.X, ALU.add)
                nc.vector.tensor_scalar_mul(havg[hf][:], havg[hf][:], 1.0 / 120.0)

        # ---- interleaved emission ----
        state = {"e2": 0, "e3": 0}

        def pump(c1_round):
            # after conv1 round c, conv2 chunks n <= 2c are safe
            n2max = min(2 * c1_round, 14)
            while state["e2"] <= n2max:
                conv2_chunk(state["e2"])
                state["e2"] += 1
                while state["e3"] < 8 and state["e2"] >= min(2 * state["e3"] + 3, 15):
                    conv3_chunk(0, state["e3"])
                    state["e3"] += 1

        for c in range(8):
            for g in range(4):
                conv1_chunk(g, c)
            pump(c)
        while state["e2"] < 15:
            conv2_chunk(state["e2"])
            state["e2"] += 1
            while state["e3"] < 8 and state["e2"] >= 2 * state["e3"] + 3:
                conv3_chunk(0, state["e3"])
                state["e3"] += 1
        while state["e3"] < 8:
            conv3_chunk(0, state["e3"])
            state["e3"] += 1
        for ci in range(8):
            conv3_chunk(1, ci)

        # ---- in_proj: M-tiles (z:0-3, xBC, dt), K=2x128 ----
        ip = pp.tile([128, 352], F32, tag="c1", bufs=2, name="ip")
        mtiles = [(10, 1152, 8), (8, 1024, 64), (9, 1088, 64)]
        mtiles += [(m, 128 * m, 128) for m in range(4, 8)]
        mtiles += [(m, 128 * m, 128) for m in range(4)]
        for m, f0, mm in mtiles:
            for k in range(2):
                nc.tensor.matmul(
                    ip[0:mm, 32 * m:32 * m + 32],
                    w_inT[:, 1160 * k + f0:1160 * k + f0 + mm],
                    havg[k][:],
                    start=(k == 0), stop=(k == 1),
                )

        # ---- mamba + classifier (feature-major, batch on free dim) ----
        def silu_act(dst, src, bias, scale):
            # silu via sigmoid * identity (no Silu table in this interp)
            v = pt.tile(list(dst.shape), F32, tag="silu_v", name="silu_v")
            s = pt.tile(list(dst.shape), F32, tag="silu_s", name="silu_s")
            nc.scalar.activation(v[:], src, AF.Identity, bias=bias, scale=scale)
            nc.scalar.activation(s[:], src, AF.Sigmoid, bias=bias, scale=scale)
            nc.vector.tensor_mul(dst, v[:], s[:])

        xcB = pt.tile([64, BSH], F32, tag="xcB")
        silu_act(xcB[:], ip[0:64, 256:288],
                 vecs[0:64, 37:38], vecs[0:64, 32:33])
        xcC = pt.tile([64, BSH], F32, tag="xcC")
        silu_act(xcC[:], ip[0:64, 288:320],
                 vecs[0:64, 43:44], vecs[0:64, 42:43])
        dts = pt.tile([8, BSH], F32, tag="dts")
        # softplus(x + b) = ln(1 + exp(x + b)) (no softplus ACT table here)
        nc.scalar.activation(
            dts[:], ip[0:8, 320:352], AF.Exp, bias=vecs[0:8, 25:26]
        )
        nc.scalar.activation(dts[:], dts[:], AF.Ln, bias=1.0)
        xc = [pt.tile([128, BSH], F32, tag=f"xc{m}", name=f"xc{m}") for m in range(4)]
        for m in range(4):
            silu_act(xc[m][:], ip[:, 32 * (4 + m):32 * (4 + m) + 32],
                     vecs[:, 33 + m:34 + m], vecs[:, 28 + m:29 + m])
        zsig = pt.tile([128, 4 * BSH], F32, tag="zsig")
        nc.scalar.activation(zsig[:], ip[:, 0:128], AF.Sigmoid)
        zsall = pt.tile([128, 4 * BSH], F32, tag="zsall")
        nc.vector.tensor_mul(zsall[:], ip[:, 0:128], zsig[:])
        zs = [zsall[:, 32 * m:32 * m + 32] for m in range(4)]

        # s8[h, b] = sum_f Bm*Cm (broadcast to 8 heads via ones lhsT)
        bc = pt.tile([64, BSH], F32, tag="bc")
        nc.vector.tensor_mul(bc[:], xcB[:], xcC[:])
        ps_s8 = pp.tile([8, BSH], F32, tag="mm", bufs=2, name="ps_s8")
        nc.tensor.matmul(ps_s8[:], ones8[:], bc[:], start=True, stop=True)
        g = pt.tile([8, BSH], F32, tag="g")
        nc.vector.tensor_mul(g[:], dts[:], ps_s8[:])
        nc.vector.tensor_scalar_add(g[:], g[:], vecs[0:8, 26:27])

        y = [pt.tile([128, BSH], F32, tag=f"y{t}", name=f"y{t}") for t in range(4)]
        ps_ms = pp.tile([1, BSH], F32, tag="c1", bufs=2, name="ps_ms")
        for t in range(4):
            ge = pp.tile([128, BSH], F32, tag="mm", bufs=2, name="ge")
            nc.tensor.matmul(ge[:], emat[:, 128 * t:128 * t + 128], g[:],
                             start=True, stop=True)
            nc.vector.tensor_mul(y[t][:], xc[t][:], ge[:])
            nc.vector.tensor_mul(y[t][:], y[t][:], zs[t])
            sq = pt.tile([128, BSH], F32, tag="sq")
            nc.vector.tensor_mul(sq[:], y[t][:], y[t][:])
            nc.tensor.matmul(ps_ms[:], ones_col[:], sq[:],
                             start=(t == 0), stop=(t == 3))
        sd = pt.tile([1, BSH], F32, tag="sd")
        nc.scalar.activation(sd[:], ps_ms[:], AF.Sqrt,
                             bias=eps_col[:], scale=1.0 / 512.0)
        rinv = pt.tile([1, BSH], F32, tag="rinv")
        nc.vector.reciprocal(rinv[:], sd[:])
        ps_rb = pp.tile([128, BSH], F32, tag="mm", bufs=2, name="ps_rb")
        nc.tensor.matmul(ps_rb[:], ones_row[:], rinv[:], start=True, stop=True)

        yn = [pt.tile([128, BSH], F32, tag=f"yn{t}", name=f"yn{t}") for t in range(4)]
        for t in range(4):
            nc.vector.tensor_mul(yn[t][:], y[t][:], ps_rb[:])
            nc.vector.tensor_scalar_mul(yn[t][:], yn[t][:],
                                        vecs[:, 38 + t:39 + t])

        # out_proj [256,512] @ yn -> o [256, 32] (2 M-tiles in one psum)
        ps_o = pp.tile([128, 64], F32, tag="mm", bufs=2, name="ps_o")
        for m in range(2):
            for k in range(4):
                nc.tensor.matmul(
                    ps_o[:, 32 * m:32 * m + 32],
                    w_outT[:, (k * 2 + m) * 128:(k * 2 + m) * 128 + 128],
                    yn[k][:],
                    start=(k == 0), stop=(k == 3),
                )
        o_sb = pt.tile([128, 64], F32, tag="o_sb")
        nc.vector.tensor_copy(o_sb[:], ps_o[:])

        # fc1 + bn4 + relu
        ps_f1 = pp.tile([64, BSH], F32, tag="c1", bufs=2, name="ps_f1")
        for k in range(2):
            nc.tensor.matmul(
                ps_f1[:], f1wT[:, 64 * k:64 * k + 64],
                o_sb[:, 32 * k:32 * k + 32],
                start=(k == 0), stop=(k == 1),
            )
        o1 = pt.tile([64, BSH], F32, tag="o1")
        nc.scalar.activation(o1[:], ps_f1[:], AF.Relu,
                             bias=c_all[0:64, 4:5], scale=s_all[0:64, 4:5])

        # fc2
        ps_f2 = pp.tile([1, BSH], F32, tag="c1", bufs=2, name="ps_f2")
        nc.tensor.matmul(ps_f2[:], f2wT[:], o1[:], start=True, stop=True)
        ores = pt.tile([1, BSH], F32, tag="ores")
        nc.scalar.activation(ores[:], ps_f2[:], AF.Identity,
                             bias=vecs[0:1, 27:28])
        nc.sync.dma_start(y_d, ores[:])


_NC_CACHE = []


def kernel(**inputs):
    if not _NC_CACHE:
        _NC_CACHE.append(_build_nc())
    nc = _NC_CACHE[0]
    w = _prep_weights(inputs)
    x = np.asarray(inputs["x"], np.float32)
    in_maps = []
    for c in range(NCORES):
        m = dict(w)
        m["x"] = np.ascontiguousarray(x[c * BSH:(c + 1) * BSH])
        in_maps.append(m)
    res = run_bass_kernel_spmd(nc, in_maps, list(range(NCORES))).results
    out = np.concatenate([res[c]["y"].reshape(BSH, 1) for c in range(NCORES)], 0)
    return out
